# revision 11
# baseline (speedup 1.0000x reference)
"""BitLinear Trainium2 kernel: LayerNorm -> x @ sign(W).T + b -> global absmax
quantize/dequantize -> * ||W||_F * sqrt(dim).

Data-parallel over the batch dim (8 batches -> 8 NeuronCores). The global
absmax over the full activation tensor is an on-device AllReduce(max).

ln_w is folded into the sign weights on the host (st = ln_w[:,None]*sign(W).T)
and ln_b into the bias row (b_eff = b + ln_b @ sign(W).T), so the device does
only (x-mu)*rsqrt(var+eps) before the matmul.

Self-contained: hardcodes shapes for x:(8,2048,4096) f32, W:(4096,4096) f32.
"""
import numpy as np
import ml_dtypes

import concourse.bass as bass
import concourse.bacc as bacc
import concourse.mybir as mybir
import concourse.tile as tile
import concourse.bass_isa as bass_isa
from concourse.bass_utils import run_bass_kernel_spmd

F32 = mybir.dt.float32
BF16 = mybir.dt.bfloat16
MAGIC = 12582912.0  # 1.5 * 2**23: adding then subtracting rounds f32 to nearest int
EPS = 1e-5

NCORES = 8
T = 2048          # tokens per core
D = 4096          # hidden dim
P = 128
NT = T // P       # 16 token tiles
KC = D // P       # 32 contraction chunks
NOUT = 512        # matmul moving free dim (= 1 PSUM bank of f32)
OC = D // NOUT    # 8 output chunks
NHALF = 2         # token-tile groups (SBUF can't hold xnT for all 16 tiles + weights)
TPH = NT // NHALF  # token tiles per group
NKEEP = 4         # last y tiles kept in SBUF (skip the DRAM round-trip in pass 2)


def _build(post_scale: float):
    nc = bacc.Bacc("TRN2", target_bir_lowering=False, debug=False,
                   num_devices=NCORES)
    xin = nc.dram_tensor("xin", [T, D], F32, kind="ExternalInput")
    st = nc.dram_tensor("st", [D, D], BF16, kind="ExternalInput")
    bvec = nc.dram_tensor("bvec", [1, D], BF16, kind="ExternalInput")
    out = nc.dram_tensor("out", [T, D], F32, kind="ExternalOutput")

    with tile.TileContext(nc) as tc:
        with (
            tc.tile_pool(name="consts", bufs=1) as consts,
            tc.tile_pool(name="dram", bufs=1, space="DRAM") as dram,
            tc.tile_pool(name="psumY", bufs=4, space="PSUM") as psumY,
            tc.tile_pool(name="xnT_pool", bufs=TPH) as xnT_pool,
        ):
            ybuf = dram.tile([T, D], F32)
            cc_in = dram.tile([1, 1], F32)
            cc_out = dram.tile([1, 1], F32, addr_space="Shared")

            ones_t = consts.tile([1, P], BF16)
            nc.vector.memset(ones_t[:], 1.0)
            b_sb = consts.tile([1, D], BF16)
            nc.sync.dma_start(b_sb[:], bvec.ap())
            amall = consts.tile([P, OC * NT], F32)
            eps_sb = consts.tile([P, 1], F32)
            nc.vector.memset(eps_sb[:], EPS)

            xnT_tiles = [None] * NT
            ykeep_tiles = {}
            with (
                tc.tile_pool(name="stp", bufs=2) as stp,
                tc.tile_pool(name="ysbp", bufs=3) as ysbp,
                tc.tile_pool(name="workA", bufs=2) as workA,
                tc.tile_pool(name="smallA", bufs=3) as smallA,
            ):
                for half in range(NHALF):
                    # ---- phase A: LayerNorm + DMA-transpose to [d, t] bf16 ----
                    for tt in range(half * TPH, (half + 1) * TPH):
                        xf = workA.tile([P, D], F32, tag="xf")
                        nc.sync.dma_start(xf[:], xin.ap()[tt * P:(tt + 1) * P, :])
                        ngroups = D // 512
                        bnout = smallA.tile([P, ngroups, 6], F32, tag="bnout")
                        for g in range(ngroups):
                            nc.vector.bn_stats(bnout[:, g, :],
                                               xf[:, g * 512:(g + 1) * 512])
                        aggr = smallA.tile([P, 2], F32, tag="aggr")
                        nc.vector.bn_aggr(aggr[:],
                                          bnout[:].rearrange("p g f -> p (g f)"))
                        std = smallA.tile([P, 1], F32, tag="std")
                        nc.scalar.activation(std[:], aggr[:, 1:2],
                                             mybir.ActivationFunctionType.Sqrt,
                                             bias=eps_sb[:])
                        rs = smallA.tile([P, 1], F32, tag="rs")
                        nc.vector.reciprocal(rs[:], std[:])
                        nmurs = smallA.tile([P, 1], F32, tag="nmurs")
                        nc.vector.tensor_scalar(nmurs[:], aggr[:, 0:1], rs[:], -1.0,
                                                mybir.AluOpType.mult,
                                                mybir.AluOpType.mult)
                        xc = workA.tile([P, D], BF16, tag="xc")
                        # (x - mu) * rs == x*rs + (-mu*rs), on the scalar engine
                        nc.scalar.activation(xc[:], xf[:],
                                             mybir.ActivationFunctionType.Identity,
                                             bias=nmurs[:], scale=rs[:])
                        xnT = xnT_pool.tile([P, KC, P], BF16, tag="xnT")
                        xnT_tiles[tt] = xnT
                        nc.scalar.dma_start_transpose(xnT[:], xc[:])

                    # ---- phase B: y = xn @ sign(W).T + b, tracking absmax ----
                    for oc in range(OC):
                        stt = stp.tile([P, KC, NOUT], BF16, tag="stt")
                        nc.sync.dma_start(
                            stt[:],
                            st.ap()[:, oc * NOUT:(oc + 1) * NOUT].rearrange(
                                "(kc p) o -> p kc o", p=P))
                        for tt in range(half * TPH, (half + 1) * TPH):
                            yp = psumY.tile([P, NOUT], F32, tag="yp")
                            for kc in range(KC):
                                nc.tensor.matmul(yp[:], xnT_tiles[tt][:, kc, :],
                                                 stt[:, kc, :],
                                                 start=(kc == 0), stop=False)
                            nc.tensor.matmul(yp[:], ones_t[:],
                                             b_sb[:, oc * NOUT:(oc + 1) * NOUT],
                                             start=False, stop=True)
                            idx = oc * NT + tt
                            nc.vector.tensor_reduce(amall[:, idx:idx + 1], yp[:],
                                                    axis=mybir.AxisListType.X,
                                                    op=mybir.AluOpType.max,
                                                    apply_absolute_value=True)
                            if tt in ykeep_tiles:
                                nc.scalar.copy(
                                    ykeep_tiles[tt][:, oc * NOUT:(oc + 1) * NOUT],
                                    yp[:])
                            else:
                                ysb = ysbp.tile([P, NOUT], F32, tag="ysb")
                                nc.scalar.copy(ysb[:], yp[:])
                                nc.sync.dma_start(
                                    ybuf[tt * P:(tt + 1) * P,
                                         oc * NOUT:(oc + 1) * NOUT], ysb[:])

            # ---- global absmax across partitions, then across cores ----
            rmax = consts.tile([P, 1], F32)
            nc.vector.tensor_reduce(rmax[:], amall[:], axis=mybir.AxisListType.X,
                                    op=mybir.AluOpType.max)
            red = consts.tile([P, 1], F32)
            nc.gpsimd.partition_all_reduce(red[:], rmax[:], channels=P,
                                           reduce_op=bass_isa.ReduceOp.max)
            nc.sync.dma_start(cc_in[:], red[0:1, :])
            nc.gpsimd.collective_compute(
                "AllReduce", mybir.AluOpType.max,
                replica_groups=[list(range(NCORES))],
                ins=[cc_in[:]], outs=[cc_out[:]])
            gm = consts.tile([1, 1], F32)
            nc.sync.dma_start(gm[:], cc_out[:])
            rcp = consts.tile([1, 1], F32)
            nc.vector.reciprocal(rcp[:], gm[:])
            sck = consts.tile([1, 2], F32)
            nc.vector.tensor_scalar_mul(sck[:, 0:1], rcp[:], 127.0)
            nc.vector.tensor_scalar_mul(sck[:, 1:2], gm[:], post_scale / 127.0)
            sckb = consts.tile([P, 2], F32)
            nc.gpsimd.partition_broadcast(sckb[:], sck[:])

            # ---- pass 2: quantize/dequantize + final scaling ----
            # step 1 (ACT): t = y*scale + MAGIC  (f32 add rounds to integer)
            # step 2 (DVE): out = (t - MAGIC) * (gm/127 * frob * sqrt(D))
            with tc.tile_pool(name="pass2", bufs=3) as pass2:
                for tt in range(NT):
                    yt = pass2.tile([P, D], F32, tag="yt")
                    nc.sync.dma_start(yt[:], ybuf[tt * P:(tt + 1) * P, :])
                    yt1 = pass2.tile([P, D], F32, tag="yt1", bufs=2)
                    nc.scalar.activation(yt1[:], yt[:],
                                         mybir.ActivationFunctionType.Copy,
                                         bias=MAGIC, scale=sckb[:, 0:1])
                    nc.vector.tensor_scalar(yt[:], yt1[:], MAGIC, sckb[:, 1:2],
                                            mybir.AluOpType.subtract,
                                            mybir.AluOpType.mult)
                    nc.scalar.dma_start(out.ap()[tt * P:(tt + 1) * P, :], yt[:])

    nc.compile()
    return nc


_CACHE = {}


def _get_nc(post_scale: float):
    key = round(float(post_scale), 6)
    if key not in _CACHE:
        _CACHE[key] = _build(post_scale)
    return _CACHE[key]


def _prep(x, ln_w, ln_b, W, b):
    x = np.asarray(x, dtype=np.float32)
    ln_w = np.asarray(ln_w, dtype=np.float32)
    ln_b = np.asarray(ln_b, dtype=np.float32)
    W = np.asarray(W, dtype=np.float32)
    b = np.asarray(b, dtype=np.float32)
    assert x.shape == (NCORES, T, D), x.shape

    frob = np.sqrt(np.sum(W.astype(np.float64) ** 2))
    post_scale = float(frob) * float(np.sqrt(np.float32(D)))

    sT = np.ascontiguousarray(np.sign(W).T)           # [d, o] f32
    st_host = (ln_w[:, None] * sT).astype(ml_dtypes.bfloat16)
    b_eff = b + ln_b @ sT                             # [o] f32
    b_host = np.ascontiguousarray(b_eff.reshape(1, D)).astype(ml_dtypes.bfloat16)

    nc = _get_nc(post_scale)
    in_maps = [
        {"xin": np.ascontiguousarray(x[c]), "st": st_host, "bvec": b_host}
        for c in range(NCORES)
    ]
    return nc, in_maps


def kernel(x, ln_w, ln_b, W, b):
    nc, in_maps = _prep(x, ln_w, ln_b, W, b)
    res = run_bass_kernel_spmd(nc, in_maps, core_ids=list(range(NCORES)))
    return np.stack([res.results[c]["out"] for c in range(NCORES)])


# Exposed for test harnesses that want profiling without rebuilding.
def run_profiled(x, ln_w, ln_b, W, b, **spmd_kwargs):
    nc, in_maps = _prep(x, ln_w, ln_b, W, b)
    res = run_bass_kernel_spmd(nc, in_maps, core_ids=list(range(NCORES)),
                               **spmd_kwargs)
    return np.stack([res.results[c]["out"] for c in range(NCORES)]), res


# revision 19
# speedup vs baseline: 1.0463x; 1.0463x over previous
"""BitLinear Trainium2 kernel: LayerNorm -> x @ sign(W).T + b -> global absmax
quantize/dequantize -> * ||W||_F * sqrt(dim).

Data-parallel over the batch dim (8 batches -> 8 NeuronCores). The global
absmax over the full activation tensor is an on-device AllReduce(max).

ln_w is folded into the sign weights on the host (st = ln_w[:,None]*sign(W).T)
and ln_b into the bias row (b_eff = b + ln_b @ sign(W).T), so the device does
only (x-mu)*rsqrt(var+eps) before the matmul.

Self-contained: hardcodes shapes for x:(8,2048,4096) f32, W:(4096,4096) f32.
"""
import numpy as np
import ml_dtypes

import concourse.bass as bass
import concourse.bacc as bacc
import concourse.mybir as mybir
import concourse.tile as tile
import concourse.bass_isa as bass_isa
from concourse import masks
from concourse.bass_utils import run_bass_kernel_spmd

F32 = mybir.dt.float32
BF16 = mybir.dt.bfloat16
F16 = mybir.dt.float16
MAGIC = 12582912.0  # 1.5 * 2**23: adding then subtracting rounds f32 to nearest int
EPS = 1e-5

NCORES = 8
T = 2048          # tokens per core
D = 4096          # hidden dim
P = 128
NT = T // P       # 16 token tiles
KC = D // P       # 32 contraction chunks
NOUT = 512        # matmul moving free dim (= 1 PSUM bank of f32)
OC = D // NOUT    # 8 output chunks
NHALF = 2         # token-tile groups (SBUF can't hold xnT for all 16 tiles + weights)
TPH = NT // NHALF  # token tiles per group
NKEEP = 4         # last y tiles kept in SBUF (skip the DRAM round-trip in pass 2)


def _build(post_scale: float):
    nc = bacc.Bacc("TRN2", target_bir_lowering=False, debug=False,
                   num_devices=NCORES)
    xin = nc.dram_tensor("xin", [T, D], F32, kind="ExternalInput")
    st = nc.dram_tensor("st", [D, D], BF16, kind="ExternalInput")
    bvec = nc.dram_tensor("bvec", [1, D], BF16, kind="ExternalInput")
    out = nc.dram_tensor("out", [T, D], F32, kind="ExternalOutput")

    with tile.TileContext(nc) as tc:
        with (
            tc.tile_pool(name="consts", bufs=1) as consts,
            tc.tile_pool(name="dram", bufs=1, space="DRAM") as dram,
            tc.tile_pool(name="psumY", bufs=4, space="PSUM") as psumY,
            tc.tile_pool(name="xnT_pool", bufs=TPH) as xnT_pool,
        ):
            ybuf = dram.tile([T, D], F16)
            cc_in = dram.tile([1, 1], F32)
            cc_out = dram.tile([1, 1], F32, addr_space="Shared")

            ones_t = consts.tile([1, P], BF16)
            nc.vector.memset(ones_t[:], 1.0)
            identf = consts.tile([P, P], F32)
            masks.make_identity(nc, identf[:])
            b_sb = consts.tile([1, D], BF16)
            nc.sync.dma_start(b_sb[:], bvec.ap())
            amall = consts.tile([P, OC * NT], F32)
            eps_sb = consts.tile([P, 1], F32)
            nc.vector.memset(eps_sb[:], EPS)

            xnT_tiles = [None] * NT
            ykeep_tiles = {}
            with (
                tc.tile_pool(name="stp", bufs=2) as stp,
                tc.tile_pool(name="ysbp", bufs=3) as ysbp,
                tc.tile_pool(name="workA", bufs=2) as workA,
                tc.tile_pool(name="smallA", bufs=3) as smallA,
            ):
                for half in range(NHALF):
                    # ---- phase A: LayerNorm + DMA-transpose to [d, t] bf16 ----
                    for tt in range(half * TPH, (half + 1) * TPH):
                        xf = workA.tile([P, D], F32, tag="xf")
                        nc.sync.dma_start(xf[:], xin.ap()[tt * P:(tt + 1) * P, :])
                        ngroups = D // 512
                        bnout = smallA.tile([P, ngroups, 6], F32, tag="bnout")
                        for g in range(ngroups):
                            nc.vector.bn_stats(bnout[:, g, :],
                                               xf[:, g * 512:(g + 1) * 512])
                        aggr = smallA.tile([P, 2], F32, tag="aggr")
                        nc.vector.bn_aggr(aggr[:],
                                          bnout[:].rearrange("p g f -> p (g f)"))
                        std = smallA.tile([P, 1], F32, tag="std")
                        nc.scalar.activation(std[:], aggr[:, 1:2],
                                             mybir.ActivationFunctionType.Sqrt,
                                             bias=eps_sb[:])
                        rs = smallA.tile([P, 1], F32, tag="rs")
                        nc.vector.reciprocal(rs[:], std[:])
                        xc = workA.tile([P, D], BF16, tag="xc")
                        nc.vector.tensor_scalar(xc[:], xf[:], aggr[:, 0:1], rs[:],
                                                mybir.AluOpType.subtract,
                                                mybir.AluOpType.mult)
                        xnT = xnT_pool.tile([P, KC, P], BF16, tag="xnT")
                        xnT_tiles[tt] = xnT
                        nc.scalar.dma_start_transpose(xnT[:], xc[:])

                    # ---- phase B: y = xn @ sign(W).T + b, tracking absmax ----
                    for oc in range(OC):
                        stt = stp.tile([P, KC, NOUT], BF16, tag="stt")
                        nc.sync.dma_start(
                            stt[:],
                            st.ap()[:, oc * NOUT:(oc + 1) * NOUT].rearrange(
                                "(kc p) o -> p kc o", p=P))
                        for tt in range(half * TPH, (half + 1) * TPH):
                            yp = psumY.tile([P, NOUT], F32, tag="yp")
                            for kc in range(KC):
                                nc.tensor.matmul(yp[:], xnT_tiles[tt][:, kc, :],
                                                 stt[:, kc, :],
                                                 start=(kc == 0), stop=False)
                            nc.tensor.matmul(yp[:], ones_t[:],
                                             b_sb[:, oc * NOUT:(oc + 1) * NOUT],
                                             start=False, stop=True)
                            idx = oc * NT + tt
                            nc.vector.tensor_reduce(amall[:, idx:idx + 1], yp[:],
                                                    axis=mybir.AxisListType.X,
                                                    op=mybir.AluOpType.max,
                                                    apply_absolute_value=True)
                            ysb = ysbp.tile([P, NOUT], F16, tag="ysb")
                            nc.scalar.copy(ysb[:], yp[:])
                            nc.sync.dma_start(
                                ybuf[tt * P:(tt + 1) * P,
                                     oc * NOUT:(oc + 1) * NOUT], ysb[:])

            # ---- global absmax across partitions, then across cores ----
            rmax = consts.tile([P, 1], F32)
            nc.vector.tensor_reduce(rmax[:], amall[:], axis=mybir.AxisListType.X,
                                    op=mybir.AluOpType.max)
            with tc.tile_pool(name="psumR", bufs=1, space="PSUM") as psumR:
                rmaxT = psumR.tile([1, P], F32)
                nc.tensor.transpose(rmaxT[:], rmax[:], identf[:])
                red = consts.tile([1, 1], F32)
                nc.vector.tensor_reduce(red[:], rmaxT[:],
                                        axis=mybir.AxisListType.X,
                                        op=mybir.AluOpType.max)
                nc.sync.dma_start(cc_in[:], red[:])
            nc.gpsimd.collective_compute(
                "AllReduce", mybir.AluOpType.max,
                replica_groups=[list(range(NCORES))],
                ins=[cc_in[:]], outs=[cc_out[:]])
            gm = consts.tile([1, 1], F32)
            nc.sync.dma_start(gm[:], cc_out[:])
            rcp = consts.tile([1, 1], F32)
            nc.vector.reciprocal(rcp[:], gm[:])
            sck = consts.tile([1, 2], F32)
            nc.vector.tensor_scalar_mul(sck[:, 0:1], rcp[:], 127.0)
            nc.vector.tensor_scalar_mul(sck[:, 1:2], gm[:], post_scale / 127.0)
            sckb = consts.tile([P, 2], F32)
            nc.gpsimd.partition_broadcast(sckb[:], sck[:])

            # ---- pass 2: quantize/dequantize + final scaling ----
            # step 1 (ACT): t = y*scale + MAGIC  (f32 add rounds to integer)
            # step 2 (DVE): out = (t - MAGIC) * (gm/127 * frob * sqrt(D))
            with tc.tile_pool(name="pass2", bufs=3) as pass2:
                for tt in range(NT):
                    ytq = pass2.tile([P, D], F16, tag="ytq")
                    nc.sync.dma_start(ytq[:], ybuf[tt * P:(tt + 1) * P, :])
                    yt1 = pass2.tile([P, D], F32, tag="yt1")
                    nc.scalar.activation(yt1[:], ytq[:],
                                         mybir.ActivationFunctionType.Copy,
                                         bias=MAGIC, scale=sckb[:, 0:1])
                    yt2 = pass2.tile([P, D], F32, tag="yt2")
                    nc.vector.tensor_scalar(yt2[:], yt1[:], MAGIC, sckb[:, 1:2],
                                            mybir.AluOpType.subtract,
                                            mybir.AluOpType.mult)
                    nc.scalar.dma_start(out.ap()[tt * P:(tt + 1) * P, :], yt2[:])

    nc.compile()
    return nc


_CACHE = {}


def _get_nc(post_scale: float):
    key = round(float(post_scale), 6)
    if key not in _CACHE:
        _CACHE[key] = _build(post_scale)
    return _CACHE[key]


def _prep(x, ln_w, ln_b, W, b):
    x = np.asarray(x, dtype=np.float32)
    ln_w = np.asarray(ln_w, dtype=np.float32)
    ln_b = np.asarray(ln_b, dtype=np.float32)
    W = np.asarray(W, dtype=np.float32)
    b = np.asarray(b, dtype=np.float32)
    assert x.shape == (NCORES, T, D), x.shape

    frob = np.sqrt(np.sum(W.astype(np.float64) ** 2))
    post_scale = float(frob) * float(np.sqrt(np.float32(D)))

    sT = np.ascontiguousarray(np.sign(W).T)           # [d, o] f32
    st_host = (ln_w[:, None] * sT).astype(ml_dtypes.bfloat16)
    b_eff = b + ln_b @ sT                             # [o] f32
    b_host = np.ascontiguousarray(b_eff.reshape(1, D)).astype(ml_dtypes.bfloat16)

    nc = _get_nc(post_scale)
    in_maps = [
        {"xin": np.ascontiguousarray(x[c]), "st": st_host, "bvec": b_host}
        for c in range(NCORES)
    ]
    return nc, in_maps


def kernel(x, ln_w, ln_b, W, b):
    nc, in_maps = _prep(x, ln_w, ln_b, W, b)
    res = run_bass_kernel_spmd(nc, in_maps, core_ids=list(range(NCORES)))
    return np.stack([res.results[c]["out"] for c in range(NCORES)])


# Exposed for test harnesses that want profiling without rebuilding.
def run_profiled(x, ln_w, ln_b, W, b, **spmd_kwargs):
    nc, in_maps = _prep(x, ln_w, ln_b, W, b)
    res = run_bass_kernel_spmd(nc, in_maps, core_ids=list(range(NCORES)),
                               **spmd_kwargs)
    return np.stack([res.results[c]["out"] for c in range(NCORES)]), res


# revision 20
# speedup vs baseline: 1.2123x; 1.1587x over previous
"""BitLinear Trainium2 kernel: LayerNorm -> x @ sign(W).T + b -> global absmax
quantize/dequantize -> * ||W||_F * sqrt(dim).

Data-parallel over the batch dim (8 batches -> 8 NeuronCores). The global
absmax over the full activation tensor is an on-device AllReduce(max).

LayerNorm is affine in x, so it is folded into the matmul instead of applied
up front:  y[t,o] = rs_t*(x@st)[t,o] - rs_t*mu_t*cs[o] + rs_t*std_t*beff[o]
with st = ln_w[:,None]*sign(W).T, cs = colsum(st), beff = b + ln_b@sign(W).T,
std_t = sqrt(var_t+eps), rs_t = 1/std_t (so rs*std ~= 1). The rank-1
correction rides on the PSUM accumulation as one extra K=2 matmul, and rs_t
is the per-partition scale of the PSUM-evacuation copy. The raw x is cast to
bf16 on the host and transposed on-chip by the DMA xbar.

Self-contained: hardcodes shapes for x:(8,2048,4096) f32, W:(4096,4096) f32.
"""
import numpy as np
import ml_dtypes

import concourse.bass as bass
import concourse.bacc as bacc
import concourse.mybir as mybir
import concourse.tile as tile
import concourse.bass_isa as bass_isa
from concourse import masks
from concourse.bass_utils import run_bass_kernel_spmd

F32 = mybir.dt.float32
BF16 = mybir.dt.bfloat16
F16 = mybir.dt.float16
MAGIC = 12582912.0  # 1.5 * 2**23: adding then subtracting rounds f32 to nearest int
EPS = 1e-5

NCORES = 8
T = 2048          # tokens per core
D = 4096          # hidden dim
P = 128
NT = T // P       # 16 token tiles
KC = D // P       # 32 contraction chunks
NOUT = 512        # matmul moving free dim (= 1 PSUM bank of f32)
OC = D // NOUT    # 8 output chunks
NHALF = 2         # token-tile groups (SBUF can't hold xnT for all 16 tiles + weights)
TPH = NT // NHALF  # token tiles per group


def _build(post_scale: float):
    nc = bacc.Bacc("TRN2", target_bir_lowering=False, debug=False,
                   num_devices=NCORES)
    xin = nc.dram_tensor("xin", [T, D], BF16, kind="ExternalInput")
    st = nc.dram_tensor("st", [D, D], BF16, kind="ExternalInput")
    csbf = nc.dram_tensor("csbf", [2, D], BF16, kind="ExternalInput")
    out = nc.dram_tensor("out", [T, D], F32, kind="ExternalOutput")

    with tile.TileContext(nc) as tc:
        with (
            tc.tile_pool(name="consts", bufs=1) as consts,
            tc.tile_pool(name="dram", bufs=1, space="DRAM") as dram,
            tc.tile_pool(name="psumY", bufs=4, space="PSUM") as psumY,
            tc.tile_pool(name="psumM", bufs=2, space="PSUM") as psumM,
            tc.tile_pool(name="xnT_pool", bufs=TPH) as xnT_pool,
            tc.tile_pool(name="rowp", bufs=TPH + 2) as rowp,
        ):
            ybuf = dram.tile([T, D], F16)
            cc_in = dram.tile([1, 1], F32)
            cc_out = dram.tile([1, 1], F32, addr_space="Shared")

            identf = consts.tile([P, P], F32)
            masks.make_identity(nc, identf[:])
            csbf_sb = consts.tile([2, D], BF16)
            nc.sync.dma_start(csbf_sb[:], csbf.ap())
            amall = consts.tile([P, OC * NT], F32)
            eps_sb = consts.tile([P, 1], F32)
            nc.vector.memset(eps_sb[:], EPS)

            xnT_tiles = [None] * NT
            row_tiles = [None] * NT
            rs_tiles = [None] * NT
            with (
                tc.tile_pool(name="stp", bufs=2) as stp,
                tc.tile_pool(name="ysbp", bufs=3) as ysbp,
                tc.tile_pool(name="workA", bufs=2) as workA,
                tc.tile_pool(name="smallA", bufs=3) as smallA,
            ):
                for half in range(NHALF):
                    # ---- phase A: load bf16 x, stats, transpose to [d, t] ----
                    for tt in range(half * TPH, (half + 1) * TPH):
                        xb = workA.tile([P, D], BF16, tag="xb")
                        nc.sync.dma_start(xb[:], xin.ap()[tt * P:(tt + 1) * P, :])
                        xnT = xnT_pool.tile([P, KC, P], BF16, tag="xnT")
                        xnT_tiles[tt] = xnT
                        nc.scalar.dma_start_transpose(xnT[:], xb[:])

                        ngroups = D // 512
                        bnout = smallA.tile([P, ngroups, 6], F32, tag="bnout")
                        for g in range(ngroups):
                            nc.vector.bn_stats(bnout[:, g, :],
                                               xb[:, g * 512:(g + 1) * 512])
                        aggr = smallA.tile([P, 2], F32, tag="aggr")
                        nc.vector.bn_aggr(aggr[:],
                                          bnout[:].rearrange("p g f -> p (g f)"))
                        # musd = [mu, std] per token; std = sqrt(var + eps)
                        musd = smallA.tile([P, 2], F32, tag="musd")
                        nc.vector.tensor_copy(musd[:, 0:1], aggr[:, 0:1])
                        nc.scalar.activation(musd[:, 1:2], aggr[:, 1:2],
                                             mybir.ActivationFunctionType.Sqrt,
                                             bias=eps_sb[:])
                        rs = rowp.tile([P, 1], F32, tag="rs")
                        rs_tiles[tt] = rs
                        nc.vector.reciprocal(rs[:], musd[:, 1:2])
                        # transpose [mu, std] to a [2, 128] bf16 row pair for
                        # the K=2 rank-1 correction matmul
                        musdT = psumM.tile([2, P], F32, tag="musdT")
                        nc.tensor.transpose(musdT[:], musd[:], identf[:])
                        row = rowp.tile([2, P], BF16, tag="row")
                        row_tiles[tt] = row
                        nc.scalar.copy(row[:], musdT[:])

                    # ---- phase B: y = rs*(x@st - mu*cs + std*beff) ----
                    for oc in range(OC):
                        stt = stp.tile([P, KC, NOUT], BF16, tag="stt")
                        nc.sync.dma_start(
                            stt[:],
                            st.ap()[:, oc * NOUT:(oc + 1) * NOUT].rearrange(
                                "(kc p) o -> p kc o", p=P))
                        for tt in range(half * TPH, (half + 1) * TPH):
                            yp = psumY.tile([P, NOUT], F32, tag="yp")
                            for kc in range(KC):
                                nc.tensor.matmul(yp[:], xnT_tiles[tt][:, kc, :],
                                                 stt[:, kc, :],
                                                 start=(kc == 0), stop=False)
                            nc.tensor.matmul(yp[:], row_tiles[tt][:],
                                             csbf_sb[:, oc * NOUT:(oc + 1) * NOUT],
                                             start=False, stop=True)
                            ysb = ysbp.tile([P, NOUT], F16, tag="ysb")
                            nc.scalar.mul(ysb[:], yp[:], rs_tiles[tt][:])
                            idx = oc * NT + tt
                            nc.vector.tensor_reduce(amall[:, idx:idx + 1], ysb[:],
                                                    axis=mybir.AxisListType.X,
                                                    op=mybir.AluOpType.max,
                                                    apply_absolute_value=True)
                            nc.sync.dma_start(
                                ybuf[tt * P:(tt + 1) * P,
                                     oc * NOUT:(oc + 1) * NOUT], ysb[:])

            # ---- global absmax across partitions, then across cores ----
            rmax = consts.tile([P, 1], F32)
            nc.vector.tensor_reduce(rmax[:], amall[:], axis=mybir.AxisListType.X,
                                    op=mybir.AluOpType.max)
            with tc.tile_pool(name="psumR", bufs=1, space="PSUM") as psumR:
                rmaxT = psumR.tile([1, P], F32)
                nc.tensor.transpose(rmaxT[:], rmax[:], identf[:])
                red = consts.tile([1, 1], F32)
                nc.vector.tensor_reduce(red[:], rmaxT[:],
                                        axis=mybir.AxisListType.X,
                                        op=mybir.AluOpType.max)
                nc.sync.dma_start(cc_in[:], red[:])
            nc.gpsimd.collective_compute(
                "AllReduce", mybir.AluOpType.max,
                replica_groups=[list(range(NCORES))],
                ins=[cc_in[:]], outs=[cc_out[:]])
            gm = consts.tile([1, 1], F32)
            nc.sync.dma_start(gm[:], cc_out[:])
            rcp = consts.tile([1, 1], F32)
            nc.vector.reciprocal(rcp[:], gm[:])
            sck = consts.tile([1, 2], F32)
            nc.vector.tensor_scalar_mul(sck[:, 0:1], rcp[:], 127.0)
            nc.vector.tensor_scalar_mul(sck[:, 1:2], gm[:], post_scale / 127.0)
            sckb = consts.tile([P, 2], F32)
            nc.gpsimd.partition_broadcast(sckb[:], sck[:])

            # ---- pass 2: quantize/dequantize + final scaling ----
            # step 1 (ACT): t = y*scale + MAGIC  (f32 add rounds to integer)
            # step 2 (DVE): out = (t - MAGIC) * (gm/127 * frob * sqrt(D))
            with tc.tile_pool(name="pass2", bufs=3) as pass2:
                for tt in range(NT):
                    ytq = pass2.tile([P, D], F16, tag="ytq")
                    nc.sync.dma_start(ytq[:], ybuf[tt * P:(tt + 1) * P, :])
                    yt1 = pass2.tile([P, D], F32, tag="yt1")
                    nc.scalar.activation(yt1[:], ytq[:],
                                         mybir.ActivationFunctionType.Copy,
                                         bias=MAGIC, scale=sckb[:, 0:1])
                    yt2 = pass2.tile([P, D], F32, tag="yt2")
                    nc.vector.tensor_scalar(yt2[:], yt1[:], MAGIC, sckb[:, 1:2],
                                            mybir.AluOpType.subtract,
                                            mybir.AluOpType.mult)
                    nc.scalar.dma_start(out.ap()[tt * P:(tt + 1) * P, :], yt2[:])

    nc.compile()
    return nc


_CACHE = {}


def _get_nc(post_scale: float):
    key = round(float(post_scale), 6)
    if key not in _CACHE:
        _CACHE[key] = _build(post_scale)
    return _CACHE[key]


def _prep(x, ln_w, ln_b, W, b):
    x = np.asarray(x, dtype=np.float32)
    ln_w = np.asarray(ln_w, dtype=np.float32)
    ln_b = np.asarray(ln_b, dtype=np.float32)
    W = np.asarray(W, dtype=np.float32)
    b = np.asarray(b, dtype=np.float32)
    assert x.shape == (NCORES, T, D), x.shape

    frob = np.sqrt(np.sum(W.astype(np.float64) ** 2))
    post_scale = float(frob) * float(np.sqrt(np.float32(D)))

    sT = np.ascontiguousarray(np.sign(W).T)           # [d, o] f32
    st_host = (ln_w[:, None] * sT).astype(ml_dtypes.bfloat16)
    # correction rows: row0 pairs with mu (-colsum(st)), row1 with std (beff)
    cs = st_host.astype(np.float64).sum(axis=0)       # matches device sum of bf16 st
    beff = b + ln_b @ sT
    csbf_host = np.stack([-cs.astype(np.float32), beff.astype(np.float32)])
    csbf_host = csbf_host.astype(ml_dtypes.bfloat16)  # [2, D]

    nc = _get_nc(post_scale)
    in_maps = [
        {"xin": x[c].astype(ml_dtypes.bfloat16), "st": st_host,
         "csbf": csbf_host}
        for c in range(NCORES)
    ]
    return nc, in_maps


def kernel(x, ln_w, ln_b, W, b):
    nc, in_maps = _prep(x, ln_w, ln_b, W, b)
    res = run_bass_kernel_spmd(nc, in_maps, core_ids=list(range(NCORES)))
    return np.stack([res.results[c]["out"] for c in range(NCORES)])


# Exposed for test harnesses that want profiling without rebuilding.
def run_profiled(x, ln_w, ln_b, W, b, **spmd_kwargs):
    nc, in_maps = _prep(x, ln_w, ln_b, W, b)
    res = run_bass_kernel_spmd(nc, in_maps, core_ids=list(range(NCORES)),
                               **spmd_kwargs)
    return np.stack([res.results[c]["out"] for c in range(NCORES)]), res
